# revision 2
# baseline (speedup 1.0000x reference)
"""Trainium2 Bass kernel for a 4-layer GINE graph encoder (GNN message passing).

Strategy (8 NeuronCores, SPMD):
  - Nodes sharded: core k owns rows [k*6250, (k+1)*6250), padded to 6272 (=49*128).
  - Edges partitioned by dst owner, sorted by dst, grouped into 128-dst
    segments; segment-sum on the tensor engine with host-built one-hot
    matrices (no scatter).
  - h[src] gathered via batched dma_gather (int16 indices, low/high table
    halves as two bases) from a replicated bf16 node table in DRAM,
    rebuilt each layer with an AllGather collective.
  - Bond encoder collapsed to a 512-row fp8 table (8^3 feature combos);
    per-edge bond vectors gathered per layer with dma_gather as well.
  - Residual/h_in kept resident in SBUF across all layers.
  - Atom embedding sums via one-hot matmuls.
  - LayerNorm rstd via a DVE-only bit-trick rsqrt; encoder LN affine folded
    into the following Linear on the host.
"""
import numpy as np
import ml_dtypes
from contextlib import ExitStack

import concourse.bass as bass
import concourse.tile as tile
from concourse import bacc, mybir
from concourse.bass_utils import run_bass_kernel_spmd
from concourse.masks import make_identity

BF16 = mybir.dt.bfloat16
FP8 = mybir.dt.float8e4
F32 = mybir.dt.float32
I32 = mybir.dt.int32
I16 = mybir.dt.int16
AF = mybir.ActivationFunctionType
ALU = mybir.AluOpType
bf = ml_dtypes.bfloat16

NCORES = 8
N, E, D, L = 50000, 160000, 512, 4
NPC = N // NCORES          # 6250 real nodes per core
NT = 49                    # node tiles per core
NPAD = NT * 128            # 6272 padded nodes per core
NTOT = NCORES * NPAD       # 50176
HALF = NTOT // 2           # 25088 (< int16 max)
NSEG = NT                  # 49 segments of 128 dst slots per core
LN_EPS = 1e-5
RSQRT_MAGIC = 0x5F3759DF

_cache = {}

# MLP groups: 13 groups of <=4 segments; gather chunks: 2 segments each
GROUPS = [list(range(g * 4, min(g * 4 + 4, NT))) for g in range(13)]
CHUNKS = [list(range(c * 2, min(c * 2 + 2, NT))) for c in range((NT + 1) // 2)]


def _host_prep(x, edge_attr, edge_index):
    """Build per-core index/one-hot arrays for the chunked gather layout."""
    x = np.asarray(x)
    ea = np.asarray(edge_attr)
    ei = np.asarray(edge_index)
    src, dst = ei[0].astype(np.int64), ei[1].astype(np.int64)
    combo_all = (ea[:, 0] * 64 + ea[:, 1] * 8 + ea[:, 2]).astype(np.int64)
    srcg = ((src // NPC) * NPAD + src % NPC).astype(np.int64)
    hf = (srcg >= HALF).astype(np.int64)

    dstc = dst // NPC
    dstl = dst % NPC
    seg = dstl // 128
    m = (dstl % 128).astype(np.int64)

    gid = (dstc * NSEG + seg) * 2 + hf
    cnt = np.bincount(gid, minlength=NCORES * NSEG * 2).reshape(
        NCORES, NSEG, 2)
    T2 = ((cnt.max(0) + 127) // 128).astype(np.int64)       # [NSEG, 2]
    empty = T2.sum(1) == 0
    T2[empty, 0] = 1

    # stream layout: per 2-seg chunk: [L(s0)][L(s1)][H(s0)][H(s1)]
    lstart = np.zeros(NSEG, np.int64)   # global tile index of seg's L tiles
    hstart = np.zeros(NSEG, np.int64)
    ch = []   # per chunk: (t0, TL, TH, [(s, l_off, h_off)])
    t = 0
    for segs in CHUNKS:
        t0 = t
        TL = int(sum(T2[s, 0] for s in segs))
        TH = int(sum(T2[s, 1] for s in segs))
        off = 0
        for s in segs:
            lstart[s] = t0 + off
            off += T2[s, 0]
        for s in segs:
            hstart[s] = t0 + off
            off += T2[s, 1]
        ch.append((t0, TL, TH))
        t += TL + TH
    TS_TOT = t
    T2MAX = max(TL + TH for _, TL, TH in ch)

    # rank edges within each (core, seg, half) group
    order = np.argsort(gid, kind="stable")
    gs = gid[order]
    starts = np.searchsorted(gs, np.arange(NCORES * NSEG * 2))
    rank = np.arange(E) - starts[gs]
    so, ho, co = seg[order], hf[order], dstc[order]
    base = np.where(ho == 0, lstart[so], hstart[so])
    gt = base + rank // 128                     # global tile
    row = rank % 128
    pos = gt * 128 + row                        # stream position

    idxh = np.zeros((NCORES, TS_TOT * 128), np.int16)
    idxe = np.zeros((NCORES, TS_TOT * 128), np.int16)
    ohe = np.zeros((NCORES, 128, TS_TOT, 128), bf)
    idxh[co, pos] = (srcg[order] - ho * HALF).astype(np.int16)
    idxe[co, pos] = combo_all[order].astype(np.int16)
    ohe[co, row, gt, m[order]] = 1

    def sb16(a):   # [NC, TS*128] -> [NC, 128, TS*8] (16-part wrap, repl x8)
        a = a.reshape(NCORES, TS_TOT * 8, 16).transpose(0, 2, 1)
        return np.ascontiguousarray(np.tile(a, (1, 8, 1)))

    xp = np.zeros((NCORES, NPAD, 9), np.int64)
    xp[:, :NPC] = x.reshape(NCORES, NPC, 9)
    oha = np.zeros((NCORES, 128, NT, 9, 128), bf)
    kk, nn, ff = np.meshgrid(np.arange(NCORES), np.arange(NPAD), np.arange(9),
                             indexing="ij")
    oha[kk.ravel(), xp.ravel(), (nn // 128).ravel(), ff.ravel(),
        (nn % 128).ravel()] = 1

    ohb = np.zeros((24, 512), bf)
    c = np.arange(512)
    ohb[(c // 64), c] = 1
    ohb[8 + (c // 8) % 8, c] = 1
    ohb[16 + c % 8, c] = 1

    return dict(T2=T2, ch=ch, lstart=lstart, hstart=hstart, TS_TOT=TS_TOT,
                T2MAX=T2MAX, idxh=sb16(idxh), idxe=sb16(idxe), ohe=ohe,
                oha=oha, ohb=ohb)


def _w_sb_layout(w):
    return np.ascontiguousarray(
        np.asarray(w, np.float32).reshape(4, 128, 512).transpose(1, 0, 2)
    ).astype(bf)


def _b_layout(b):
    return np.ascontiguousarray(
        np.asarray(b, np.float32).reshape(4, 128).T).astype(np.float32)


def _repl(v):
    return np.ascontiguousarray(
        np.broadcast_to(np.asarray(v, np.float32), (128, 512)))


def build_program(P, ln_ident):
    T2, ch, lstart, hstart = P["T2"], P["ch"], P["lstart"], P["hstart"]
    TS_TOT, T2MAX = P["TS_TOT"], P["T2MAX"]

    nc = bacc.Bacc("TRN2", target_bir_lowering=False, debug=False,
                   num_devices=NCORES)

    def din(name, shape, dt):
        return nc.dram_tensor(name, shape, dt, kind="ExternalInput")

    idxh = din("idxh", [128, TS_TOT * 8], I16)
    idxe = din("idxe", [128, TS_TOT * 8], I16)
    ohe = din("ohe", [128, TS_TOT, 128], BF16)
    oha = din("oha", [128, NT, 9, 128], BF16)
    ohb = din("ohb", [24, 512], BF16)
    atom_emb = din("atom_emb", [128, 9, 512], BF16)
    bond_emb = din("bond_emb", [24, 512], BF16)
    aw1 = din("aw1", [128, 4, 512], BF16)
    aw2 = din("aw2", [128, 4, 512], BF16)
    bw1 = din("bw1", [128, 4, 512], BF16)
    bw2 = din("bw2", [128, 4, 512], BF16)
    cw1 = din("cw1", [L, 128, 4, 512], BF16)
    cw2 = din("cw2", [L, 128, 4, 512], BF16)
    ab1 = din("ab1", [128, 4], F32)
    ab2 = din("ab2", [128, 4], F32)
    bb1 = din("bb1", [128, 4], F32)
    bb2 = din("bb2", [128, 4], F32)
    cb1 = din("cb1", [L, 128, 4], F32)
    cb2 = din("cb2", [L, 128, 4], F32)
    cln = din("cln", [L, 2, 128, 512], F32)

    out_h = nc.dram_tensor("out_h", [NPAD, 512], F32, kind="ExternalOutput")

    shard = [nc.dram_tensor(f"shard{i}", [NPAD, 512], BF16) for i in range(2)]
    h_tab = [nc.dram_tensor(f"h_tab{i}", [NTOT, 512], BF16,
                            addr_space="Shared") for i in range(2)]
    e_table = nc.dram_tensor("e_table", [512, 512], FP8)

    RG = [list(range(NCORES))]

    with tile.TileContext(nc) as tc:
        with ExitStack() as ctx:
            const = ctx.enter_context(tc.tile_pool(name="const", bufs=1))
            sb = ctx.enter_context(tc.tile_pool(name="sb", bufs=2))
            wpool = ctx.enter_context(tc.tile_pool(name="wpool", bufs=2))
            hres = ctx.enter_context(tc.tile_pool(name="hres", bufs=1))
            hg = ctx.enter_context(tc.tile_pool(name="hg", bufs=2))
            eg = ctx.enter_context(tc.tile_pool(name="eg", bufs=2))
            og = ctx.enter_context(tc.tile_pool(name="og", bufs=2))
            p_acc = ctx.enter_context(
                tc.tile_pool(name="p_acc", bufs=2, space="PSUM"))
            p_zt = ctx.enter_context(
                tc.tile_pool(name="p_zt", bufs=2, space="PSUM"))
            p_mm = ctx.enter_context(
                tc.tile_pool(name="p_mm", bufs=2, space="PSUM"))
            p_z2 = ctx.enter_context(
                tc.tile_pool(name="p_z2", bufs=2, space="PSUM"))

            ident = const.tile([128, 128], BF16)
            make_identity(nc, ident[:])

            _cc = [0]

            def load_const(ap, shape, dt):
                _cc[0] += 1
                t = const.tile(shape, dt, tag=f"const{_cc[0]}",
                               name=f"const{_cc[0]}")
                nc.sync.dma_start(t[:], ap)
                return t

            atom_emb_s = load_const(atom_emb[:], [128, 9, 512], BF16)
            bond_emb_s = load_const(bond_emb[:], [24, 512], BF16)
            ohb_s = load_const(ohb[:], [24, 512], BF16)
            aw1_s = load_const(aw1[:], [128, 4, 512], BF16)
            aw2_s = load_const(aw2[:], [128, 4, 512], BF16)
            bw1_s = load_const(bw1[:], [128, 4, 512], BF16)
            bw2_s = load_const(bw2[:], [128, 4, 512], BF16)
            ab1_s = load_const(ab1[:], [128, 4], F32)
            ab2_s = load_const(ab2[:], [128, 4], F32)
            bb1_s = load_const(bb1[:], [128, 4], F32)
            bb2_s = load_const(bb2[:], [128, 4], F32)
            cb1_s = [load_const(cb1[l], [128, 4], F32) for l in range(L)]
            cb2_s = [load_const(cb2[l], [128, 4], F32) for l in range(L)]
            if not ln_ident:
                cln_g_s = [load_const(cln[l, 0], [128, 512], F32)
                           for l in range(L)]
                cln_b_s = [load_const(cln[l, 1], [128, 512], F32)
                           for l in range(L)]
            idxh_s = load_const(idxh[:], [128, TS_TOT * 8], I16)
            idxe_s = load_const(idxe[:], [128, TS_TOT * 8], I16)

            hin = {}   # persistent per-seg residual tiles

            def mlp_block(rows, W, w1_s, b1_s, w2_s, b2_s, act1, evac):
                nt = W // 128
                zT = sb.tile([128, 4, W], BF16, tag=f"mT{W}", bufs=4)
                for d in range(4):
                    ztp = p_zt.tile([128, W], BF16, tag="ztp")
                    for s in range(nt):
                        nc.tensor.transpose(ztp[:, s * 128:(s + 1) * 128],
                                            rows[s][:, d * 128:(d + 1) * 128],
                                            ident[:])
                    nc.scalar.activation(zT[:, d, :], ztp[:], AF.Copy)
                a1 = sb.tile([128, 4, W], BF16, tag=f"mT{W}", bufs=4)
                for mc in range(4):
                    mm = p_mm.tile([128, W], F32, tag="mm")
                    for kc in range(4):
                        nc.tensor.matmul(mm[:],
                                         w1_s[:, kc, mc * 128:(mc + 1) * 128],
                                         zT[:, kc, :],
                                         start=(kc == 0), stop=(kc == 3))
                    nc.scalar.activation(a1[:, mc, :], mm[:], act1,
                                         bias=b1_s[:, mc:mc + 1])
                z2T = sb.tile([128, 4, W], BF16, tag=f"mT{W}", bufs=4)
                for mc in range(4):
                    mm = p_mm.tile([128, W], F32, tag="mm")
                    for kc in range(4):
                        nc.tensor.matmul(mm[:],
                                         w2_s[:, kc, mc * 128:(mc + 1) * 128],
                                         a1[:, kc, :],
                                         start=(kc == 0), stop=(kc == 3))
                    nc.scalar.activation(z2T[:, mc, :], mm[:], AF.Identity,
                                         bias=b2_s[:, mc:mc + 1])
                for s in range(nt):
                    z2p = p_z2.tile([128, 512], BF16, tag="z2p")
                    for d in range(4):
                        nc.tensor.transpose(z2p[:, d * 128:(d + 1) * 128],
                                            z2T[:, d, s * 128:(s + 1) * 128],
                                            ident[:])
                    evac(s, z2p)

            def rstd_nmrs(rsum, ssq, G):
                """LayerNorm 1/std and -mean/std via DVE-only fast rsqrt."""
                mean = sb.tile([128, 4], F32, tag="mean")
                nc.vector.tensor_scalar_mul(mean[:, :G], rsum[:, :G], 1.0 / 512)
                t1 = sb.tile([128, 4], F32, tag="t1")
                nc.vector.tensor_scalar(t1[:, :G], ssq[:, :G], 1.0 / 512,
                                        LN_EPS, op0=ALU.mult, op1=ALU.add)
                m2 = sb.tile([128, 4], F32, tag="m2")
                nc.vector.tensor_mul(m2[:, :G], mean[:, :G], mean[:, :G])
                v = sb.tile([128, 4], F32, tag="v")
                nc.vector.scalar_tensor_tensor(v[:, :G], m2[:, :G], -1.0,
                                               t1[:, :G],
                                               op0=ALU.mult, op1=ALU.add)
                vh = sb.tile([128, 4], F32, tag="vh")
                nc.vector.tensor_scalar_mul(vh[:, :G], v[:, :G], 0.5)
                y0 = sb.tile([128, 4], F32, tag="y0")
                nc.vector.tensor_scalar(
                    y0[:, :G].bitcast(I32), v[:, :G].bitcast(I32), 1, None,
                    op0=ALU.arith_shift_right)
                nc.vector.tensor_scalar(
                    y0[:, :G].bitcast(I32), y0[:, :G].bitcast(I32),
                    -1, RSQRT_MAGIC, op0=ALU.mult, op1=ALU.add)
                s2 = sb.tile([128, 4], F32, tag="s2")
                nc.vector.tensor_mul(s2[:, :G], y0[:, :G], y0[:, :G])
                u = sb.tile([128, 4], F32, tag="u")
                nc.vector.tensor_mul(u[:, :G], s2[:, :G], vh[:, :G])
                a = sb.tile([128, 4], F32, tag="a")
                nc.vector.tensor_mul(a[:, :G], y0[:, :G], u[:, :G])
                b15 = sb.tile([128, 4], F32, tag="b15")
                nc.vector.tensor_scalar_mul(b15[:, :G], y0[:, :G], 1.5)
                rstd = sb.tile([128, 4], F32, tag="rstd")
                nc.vector.scalar_tensor_tensor(rstd[:, :G], a[:, :G], -1.0,
                                               b15[:, :G],
                                               op0=ALU.mult, op1=ALU.add)
                nmrs = sb.tile([128, 4], F32, tag="nmrs")
                nc.vector.scalar_tensor_tensor(nmrs[:, :G], mean[:, :G], -1.0,
                                               rstd[:, :G],
                                               op0=ALU.mult, op1=ALU.mult)
                return rstd, nmrs

            # ================= PHASE 1: bond table (fp8) =================
            bond_rows = []
            rsum_b = sb.tile([128, 4], F32, tag="rsum")
            ssq_b = sb.tile([128, 4], F32, tag="ssq")
            for t in range(4):
                acc = p_acc.tile([128, 512], F32, tag="acc")
                nc.tensor.matmul(acc[:], ohb_s[:, t * 128:(t + 1) * 128],
                                 bond_emb_s[:], start=True, stop=True)
                rows = sb.tile([128, 512], F32, tag="rows32", bufs=6)
                nc.scalar.activation(rows[:], acc[:], AF.Identity,
                                     accum_out=rsum_b[:, t:t + 1])
                sq = sb.tile([128, 512], BF16, tag="sq")
                nc.vector.scalar_tensor_tensor(sq[:], rows[:], 1.0, rows[:],
                                               op0=ALU.bypass, op1=ALU.mult,
                                               accum_out=ssq_b[:, t:t + 1])
                bond_rows.append(rows)

            rstd_b, nmrs_b = rstd_nmrs(rsum_b, ssq_b, 4)
            xhat_b = []
            for i in range(4):
                xh = sb.tile([128, 512], BF16, tag="ln16", bufs=6,
                             name=f"bxh{i}")
                nc.scalar.activation(xh[:], bond_rows[i][:], AF.Identity,
                                     scale=rstd_b[:, i:i + 1],
                                     bias=nmrs_b[:, i:i + 1])
                xhat_b.append(xh)

            def bond_evac(s, z2p):
                eout = sb.tile([128, 512], FP8, tag="eout")
                nc.scalar.activation(eout[:], z2p[:], AF.Copy)
                nc.sync.dma_start(e_table[s * 128:(s + 1) * 128, :], eout[:])

            mlp_block(xhat_b, 512, bw1_s, bb1_s, bw2_s, bb2_s, AF.Gelu,
                      bond_evac)

            # ================= PHASE 2: atom encoder =================
            for grp in GROUPS:
                W = len(grp) * 128
                G = len(grp)
                rsum = sb.tile([128, 4], F32, tag="rsum")
                ssq = sb.tile([128, 4], F32, tag="ssq")
                rows_f = []
                for i, t in enumerate(grp):
                    oh = sb.tile([128, 9, 128], BF16, tag="oha", bufs=2)
                    nc.sync.dma_start(oh[:], oha[:, t, :, :])
                    acc = p_acc.tile([128, 512], F32, tag="acc")
                    for f in range(9):
                        nc.tensor.matmul(acc[:], oh[:, f, :],
                                         atom_emb_s[:, f, :],
                                         start=(f == 0), stop=(f == 8))
                    rows = sb.tile([128, 512], F32, tag="rows32", bufs=6)
                    nc.scalar.activation(rows[:], acc[:], AF.Identity,
                                         accum_out=rsum[:, i:i + 1])
                    sq = sb.tile([128, 512], BF16, tag="sq")
                    nc.vector.scalar_tensor_tensor(
                        sq[:], rows[:], 1.0, rows[:],
                        op0=ALU.bypass, op1=ALU.mult,
                        accum_out=ssq[:, i:i + 1])
                    rows_f.append(rows)
                rstd, nmrs = rstd_nmrs(rsum, ssq, G)
                lnr = []
                for i in range(G):
                    xh = sb.tile([128, 512], BF16, tag="ln16", bufs=6,
                                 name=f"axh{i}")
                    nc.scalar.activation(xh[:], rows_f[i][:], AF.Identity,
                                         scale=rstd[:, i:i + 1],
                                         bias=nmrs[:, i:i + 1])
                    lnr.append(xh)

                def atom_evac(i, z2p, grp=grp):
                    t = grp[i]
                    ht = hres.tile([128, 512], BF16, tag=f"hin{t}",
                                   name=f"hin{t}")
                    hin[t] = ht
                    nc.vector.tensor_copy(ht[:], z2p[:])
                    nc.sync.dma_start(shard[0][t * 128:(t + 1) * 128, :],
                                      ht[:])

                mlp_block(lnr, W, aw1_s, ab1_s, aw2_s, ab2_s, AF.Gelu,
                          atom_evac)
            nc.gpsimd.collective_compute(
                "AllGather", ALU.bypass, replica_groups=RG,
                ins=[shard[0][:]], outs=[h_tab[0][:]])

            # ================= PHASE 3: conv layers =================
            NCH = len(CHUNKS)
            for l in range(L):
                tab = h_tab[l % 2]
                shd = shard[(l + 1) % 2]
                w1_s = wpool.tile([128, 4, 512], BF16, tag="w1")
                nc.sync.dma_start(w1_s[:], cw1[l])
                w2_s = wpool.tile([128, 4, 512], BF16, tag="w2")
                nc.sync.dma_start(w2_s[:], cw2[l])

                pend = {}

                def issue_chunk(c, pend=pend, tab=tab):
                    t0, TL, TH = ch[c]
                    TC = TL + TH
                    hb = hg.tile([128, T2MAX, 512], BF16, tag="hb")
                    if TL:
                        nc.gpsimd.dma_gather(
                            hb[:, :TL, :], tab[:HALF, :],
                            idxh_s[:, t0 * 8:(t0 + TL) * 8],
                            TL * 128, TL * 128, 512)
                    if TH:
                        nc.gpsimd.dma_gather(
                            hb[:, TL:TC, :], tab[HALF:, :],
                            idxh_s[:, (t0 + TL) * 8:(t0 + TC) * 8],
                            TH * 128, TH * 128, 512)
                    eb = eg.tile([128, T2MAX, 512], FP8, tag="eb")
                    nc.gpsimd.dma_gather(
                        eb[:, :TC, :], e_table[:],
                        idxe_s[:, t0 * 8:(t0 + TC) * 8],
                        TC * 128, TC * 128, 512)
                    oh = og.tile([128, T2MAX, 128], BF16, tag="oh")
                    nc.sync.dma_start(oh[:, :TC, :], ohe[:, t0:t0 + TC, :])
                    pend[c] = (hb, eb, oh)

                issue_chunk(0)
                zs = []
                for c in range(NCH):
                    if c + 1 < NCH:
                        issue_chunk(c + 1)
                    hb, eb, oh = pend.pop(c)
                    t0, TL, TH = ch[c]
                    TC = TL + TH
                    nc.vector.tensor_add(hb[:, :TC, :], hb[:, :TC, :],
                                         eb[:, :TC, :])
                    nc.vector.tensor_scalar_max(hb[:, :TC, :], hb[:, :TC, :],
                                                0.0)
                    for s in CHUNKS[c]:
                        tl = int(lstart[s] - t0)
                        th = int(hstart[s] - t0)
                        tls = list(range(tl, tl + int(T2[s, 0]))) + \
                            list(range(th, th + int(T2[s, 1])))
                        agg = p_acc.tile([128, 512], F32, tag="acc")
                        for i, tt in enumerate(tls):
                            nc.tensor.matmul(agg[:], oh[:, tt, :],
                                             hb[:, tt, :],
                                             start=(i == 0),
                                             stop=(i == len(tls) - 1))
                        z = sb.tile([128, 512], BF16, tag="ln16", bufs=6)
                        nc.vector.tensor_add(z[:], agg[:], hin[s][:])
                        zs.append((s, z))

                    if len(zs) >= 4 or c == NCH - 1:
                        grp = [s for s, _ in zs]
                        z_rows = [zz for _, zz in zs]
                        zs = []
                        G = len(grp)
                        W = G * 128
                        rsum = sb.tile([128, 4], F32, tag="rsum")
                        ssq = sb.tile([128, 4], F32, tag="ssq")
                        r_tiles = []

                        def conv_evac(i, z2p, grp=grp, rsum=rsum, ssq=ssq,
                                      r_tiles=r_tiles):
                            g2 = sb.tile([128, 512], F32, tag="g2", bufs=2)
                            nc.scalar.activation(g2[:], z2p[:], AF.Gelu)
                            r = sb.tile([128, 512], F32, tag="rows32", bufs=6)
                            nc.vector.scalar_tensor_tensor(
                                r[:], g2[:], 0.0, hin[grp[i]][:],
                                op0=ALU.bypass, op1=ALU.add,
                                accum_out=rsum[:, i:i + 1])
                            sq = sb.tile([128, 512], BF16, tag="sq")
                            nc.vector.scalar_tensor_tensor(
                                sq[:], r[:], 1.0, r[:],
                                op0=ALU.bypass, op1=ALU.mult,
                                accum_out=ssq[:, i:i + 1])
                            r_tiles.append(r)

                        mlp_block(z_rows, W, w1_s, cb1_s[l], w2_s,
                                  cb2_s[l], AF.Relu, conv_evac)

                        rstd, nmrs = rstd_nmrs(rsum, ssq, G)
                        for i, s in enumerate(grp):
                            rs = slice(s * 128, (s + 1) * 128)
                            if ln_ident:
                                if l == L - 1:
                                    xn = sb.tile([128, 512], F32, tag="xn")
                                    nc.scalar.activation(
                                        xn[:], r_tiles[i][:], AF.Identity,
                                        scale=rstd[:, i:i + 1],
                                        bias=nmrs[:, i:i + 1])
                                    nc.sync.dma_start(out_h[rs, :], xn[:])
                                else:
                                    nc.scalar.activation(
                                        hin[s][:], r_tiles[i][:], AF.Identity,
                                        scale=rstd[:, i:i + 1],
                                        bias=nmrs[:, i:i + 1])
                                    nc.sync.dma_start(shd[rs, :], hin[s][:])
                                continue
                            xn = sb.tile([128, 512], F32, tag="xn")
                            nc.scalar.activation(xn[:], r_tiles[i][:],
                                                 AF.Identity,
                                                 scale=rstd[:, i:i + 1],
                                                 bias=nmrs[:, i:i + 1])
                            y = sb.tile([128, 512], F32, tag="y")
                            nc.vector.tensor_mul(y[:], xn[:], cln_g_s[l][:])
                            hf_ = sb.tile([128, 512], F32, tag="hf")
                            nc.vector.tensor_add(hf_[:], y[:], cln_b_s[l][:])
                            if l == L - 1:
                                nc.sync.dma_start(out_h[rs, :], hf_[:])
                            else:
                                nc.vector.tensor_copy(hin[s][:], hf_[:])
                                nc.sync.dma_start(shd[rs, :], hin[s][:])
                if l < L - 1:
                    nc.gpsimd.collective_compute(
                        "AllGather", ALU.bypass, replica_groups=RG,
                        ins=[shd[:]], outs=[h_tab[(l + 1) % 2][:]])

    nc.compile()
    return nc


def kernel(x, edge_attr, edge_index,
           atom_emb, atom_ln_g, atom_ln_b, atom_w1, atom_b1, atom_w2, atom_b2,
           bond_emb, bond_ln_g, bond_ln_b, bond_w1, bond_b1, bond_w2, bond_b2,
           conv_w1, conv_b1, conv_w2, conv_b2, ln_g, ln_b):
    prep = _host_prep(x, edge_attr, edge_index)

    ln_ident = bool(np.all(np.asarray(ln_g) == 1.0)
                    and np.all(np.asarray(ln_b) == 0.0))
    key = (prep["TS_TOT"], tuple(prep["T2"].ravel().tolist()), ln_ident)
    if key not in _cache:
        _cache[key] = build_program(prep, ln_ident)
    nc = _cache[key]

    # fold the encoder LayerNorm affine into the first Linear
    f32 = np.float32
    aw1_abs = np.asarray(atom_ln_g, f32)[:, None] * np.asarray(atom_w1, f32)
    ab1_abs = np.asarray(atom_b1, f32) + \
        np.asarray(atom_ln_b, f32) @ np.asarray(atom_w1, f32)
    bw1_abs = np.asarray(bond_ln_g, f32)[:, None] * np.asarray(bond_w1, f32)
    bb1_abs = np.asarray(bond_b1, f32) + \
        np.asarray(bond_ln_b, f32) @ np.asarray(bond_w1, f32)

    shared = dict(
        ohb=prep["ohb"],
        atom_emb=np.ascontiguousarray(
            np.asarray(atom_emb, f32).transpose(1, 0, 2)).astype(bf),
        bond_emb=np.asarray(bond_emb, f32).reshape(24, 512).astype(bf),
        aw1=_w_sb_layout(aw1_abs), aw2=_w_sb_layout(atom_w2),
        bw1=_w_sb_layout(bw1_abs), bw2=_w_sb_layout(bond_w2),
        cw1=np.stack([_w_sb_layout(conv_w1[l]) for l in range(L)]),
        cw2=np.stack([_w_sb_layout(conv_w2[l]) for l in range(L)]),
        ab1=_b_layout(ab1_abs), ab2=_b_layout(atom_b2),
        bb1=_b_layout(bb1_abs), bb2=_b_layout(bond_b2),
        cb1=np.stack([_b_layout(conv_b1[l]) for l in range(L)]),
        cb2=np.stack([_b_layout(conv_b2[l]) for l in range(L)]),
        cln=np.stack([np.stack([_repl(ln_g[l]), _repl(ln_b[l])])
                      for l in range(L)]),
    )
    in_maps = []
    for k in range(NCORES):
        m = dict(shared)
        m["idxh"] = prep["idxh"][k]
        m["idxe"] = prep["idxe"][k]
        m["ohe"] = prep["ohe"][k]
        m["oha"] = prep["oha"][k]
        in_maps.append(m)

    res = run_bass_kernel_spmd(nc, in_maps, list(range(NCORES)))
    kernel._last_results = res
    out = np.empty((N, D), np.float32)
    for k in range(NCORES):
        out[k * NPC:(k + 1) * NPC] = np.asarray(
            res.results[k]["out_h"], np.float32)[:NPC]
    return out


# revision 11
# speedup vs baseline: 1.1837x; 1.1837x over previous
"""Trainium2 Bass kernel for a 4-layer GINE graph encoder (GNN message passing).

Strategy (8 NeuronCores, SPMD):
  - Nodes sharded: core k owns rows [k*6250, (k+1)*6250), padded to 6272 (=49*128).
  - Edges partitioned by dst owner, sorted by dst, grouped into 128-dst
    segments; segment-sum on the tensor engine with host-built one-hot
    matrices (no scatter).
  - h[src] gathered via batched dma_gather (int16 indices, low/high table
    halves as two bases) from a replicated bf16 node table in DRAM,
    rebuilt each layer with an AllGather collective.
  - Bond encoder collapsed to a 512-row fp8 table (8^3 feature combos);
    per-edge bond vectors gathered per layer with dma_gather as well.
  - Residual/h_in kept resident in SBUF across all layers.
  - Atom embedding sums via one-hot matmuls.
  - LayerNorm rstd via a DVE-only bit-trick rsqrt; encoder LN affine folded
    into the following Linear on the host.
"""
import numpy as np
import ml_dtypes
from contextlib import ExitStack

import concourse.bass as bass
import concourse.tile as tile
from concourse import bacc, mybir
from concourse.bass_utils import run_bass_kernel_spmd
from concourse.masks import make_identity

BF16 = mybir.dt.bfloat16
FP8 = mybir.dt.float8e4
F32 = mybir.dt.float32
I32 = mybir.dt.int32
I16 = mybir.dt.int16
AF = mybir.ActivationFunctionType
ALU = mybir.AluOpType
bf = ml_dtypes.bfloat16

NCORES = 8
N, E, D, L = 50000, 160000, 512, 4
NPC = N // NCORES          # 6250 real nodes per core
NT = 49                    # node tiles per core
NPAD = NT * 128            # 6272 padded nodes per core
NTOT = NCORES * NPAD       # 50176
HALF = NTOT // 2           # 25088 (< int16 max)
NSEG = NT                  # 49 segments of 128 dst slots per core
LN_EPS = 1e-5
RSQRT_MAGIC = 0x5F3759DF

_cache = {}

# MLP groups: 13 groups of <=4 segments; gather chunks: 2 segments each
GROUPS = [list(range(g * 4, min(g * 4 + 4, NT))) for g in range(13)]
CHUNKS = [list(range(c * 2, min(c * 2 + 2, NT))) for c in range((NT + 1) // 2)]


def _host_prep(x, edge_attr, edge_index):
    """Build per-core index/one-hot arrays for the chunked gather layout."""
    x = np.asarray(x)
    ea = np.asarray(edge_attr)
    ei = np.asarray(edge_index)
    src, dst = ei[0].astype(np.int64), ei[1].astype(np.int64)
    combo_all = (ea[:, 0] * 64 + ea[:, 1] * 8 + ea[:, 2]).astype(np.int64)
    srcg = ((src // NPC) * NPAD + src % NPC).astype(np.int64)
    hf = (srcg >= HALF).astype(np.int64)

    dstc = dst // NPC
    dstl = dst % NPC
    seg = dstl // 128
    m = (dstl % 128).astype(np.int64)

    gid = (dstc * NSEG + seg) * 2 + hf
    cnt = np.bincount(gid, minlength=NCORES * NSEG * 2).reshape(
        NCORES, NSEG, 2)
    T2 = ((cnt.max(0) + 127) // 128).astype(np.int64)       # [NSEG, 2]
    empty = T2.sum(1) == 0
    T2[empty, 0] = 1

    # stream layout: per 2-seg chunk: [L(s0)][L(s1)][H(s0)][H(s1)]
    lstart = np.zeros(NSEG, np.int64)   # global tile index of seg's L tiles
    hstart = np.zeros(NSEG, np.int64)
    ch = []   # per chunk: (t0, TL, TH, [(s, l_off, h_off)])
    t = 0
    for segs in CHUNKS:
        t0 = t
        TL = int(sum(T2[s, 0] for s in segs))
        TH = int(sum(T2[s, 1] for s in segs))
        off = 0
        for s in segs:
            lstart[s] = t0 + off
            off += T2[s, 0]
        for s in segs:
            hstart[s] = t0 + off
            off += T2[s, 1]
        ch.append((t0, TL, TH))
        t += TL + TH
    TS_TOT = t
    T2MAX = max(TL + TH for _, TL, TH in ch)

    # rank edges within each (core, seg, half) group
    order = np.argsort(gid, kind="stable")
    gs = gid[order]
    starts = np.searchsorted(gs, np.arange(NCORES * NSEG * 2))
    rank = np.arange(E) - starts[gs]
    so, ho, co = seg[order], hf[order], dstc[order]
    base = np.where(ho == 0, lstart[so], hstart[so])
    gt = base + rank // 128                     # global tile
    row = rank % 128
    pos = gt * 128 + row                        # stream position

    idxh = np.zeros((NCORES, TS_TOT * 128), np.int16)
    idxe = np.zeros((NCORES, TS_TOT * 128), np.int16)
    ohe = np.zeros((NCORES, 128, TS_TOT, 128), bf)
    idxh[co, pos] = (srcg[order] - ho * HALF).astype(np.int16)
    idxe[co, pos] = combo_all[order].astype(np.int16)
    ohe[co, row, gt, m[order]] = 1

    def sb16(a):   # [NC, TS*128] -> [NC, 128, TS*8] (16-part wrap, repl x8)
        a = a.reshape(NCORES, TS_TOT * 8, 16).transpose(0, 2, 1)
        return np.ascontiguousarray(np.tile(a, (1, 8, 1)))

    xp = np.zeros((NCORES, NPAD, 9), np.int64)
    xp[:, :NPC] = x.reshape(NCORES, NPC, 9)
    oha = np.zeros((NCORES, 128, NT, 9, 128), bf)
    kk, nn, ff = np.meshgrid(np.arange(NCORES), np.arange(NPAD), np.arange(9),
                             indexing="ij")
    oha[kk.ravel(), xp.ravel(), (nn // 128).ravel(), ff.ravel(),
        (nn % 128).ravel()] = 1

    ohb = np.zeros((24, 512), bf)
    c = np.arange(512)
    ohb[(c // 64), c] = 1
    ohb[8 + (c // 8) % 8, c] = 1
    ohb[16 + c % 8, c] = 1

    return dict(T2=T2, ch=ch, lstart=lstart, hstart=hstart, TS_TOT=TS_TOT,
                T2MAX=T2MAX, idxh=sb16(idxh), idxe=sb16(idxe), ohe=ohe,
                oha=oha, ohb=ohb)


def _w_sb_layout(w):
    return np.ascontiguousarray(
        np.asarray(w, np.float32).reshape(4, 128, 512).transpose(1, 0, 2)
    ).astype(bf)


def _b_layout(b):
    return np.ascontiguousarray(
        np.asarray(b, np.float32).reshape(4, 128).T).astype(np.float32)


def _repl(v):
    return np.ascontiguousarray(
        np.broadcast_to(np.asarray(v, np.float32), (128, 512)))


def build_program(P, ln_ident):
    T2, ch, lstart, hstart = P["T2"], P["ch"], P["lstart"], P["hstart"]
    TS_TOT, T2MAX = P["TS_TOT"], P["T2MAX"]

    nc = bacc.Bacc("TRN2", target_bir_lowering=False, debug=False,
                   num_devices=NCORES, num_swdge_queues=4)

    def din(name, shape, dt):
        return nc.dram_tensor(name, shape, dt, kind="ExternalInput")

    idxh = din("idxh", [128, TS_TOT * 8], I16)
    idxe = din("idxe", [128, TS_TOT * 8], I16)
    ohe = din("ohe", [128, TS_TOT, 128], BF16)
    oha = din("oha", [128, NT, 9, 128], BF16)
    ohb = din("ohb", [24, 512], BF16)
    atom_emb = din("atom_emb", [128, 9, 512], BF16)
    bond_emb = din("bond_emb", [24, 512], BF16)
    aw1 = din("aw1", [128, 4, 512], BF16)
    aw2 = din("aw2", [128, 4, 512], BF16)
    bw1 = din("bw1", [128, 4, 512], BF16)
    bw2 = din("bw2", [128, 4, 512], BF16)
    cw1 = din("cw1", [L, 128, 4, 512], BF16)
    cw2 = din("cw2", [L, 128, 4, 512], BF16)
    ab1 = din("ab1", [128, 4], F32)
    ab2 = din("ab2", [128, 4], F32)
    bb1 = din("bb1", [128, 4], F32)
    bb2 = din("bb2", [128, 4], F32)
    cb1 = din("cb1", [L, 128, 4], F32)
    cb2 = din("cb2", [L, 128, 4], F32)
    cln = din("cln", [L, 2, 128, 512], F32)

    out_h = nc.dram_tensor("out_h", [NPAD, 512], F32, kind="ExternalOutput")

    shard = [nc.dram_tensor(f"shard{i}", [NPAD, 512], BF16) for i in range(2)]
    h_tab = [nc.dram_tensor(f"h_tab{i}", [NTOT, 512], BF16,
                            addr_space="Shared") for i in range(2)]
    e_table = nc.dram_tensor("e_table", [512, 512], FP8)
    e_edges = nc.dram_tensor("e_edges", [128, TS_TOT, 512], FP8)

    RG = [list(range(NCORES))]

    with tile.TileContext(nc) as tc:
        with ExitStack() as ctx:
            const = ctx.enter_context(tc.tile_pool(name="const", bufs=1))
            sb = ctx.enter_context(tc.tile_pool(name="sb", bufs=2))
            wpool = ctx.enter_context(tc.tile_pool(name="wpool", bufs=2))
            hres = ctx.enter_context(tc.tile_pool(name="hres", bufs=1))
            hg = ctx.enter_context(tc.tile_pool(name="hg", bufs=3))
            eg = ctx.enter_context(tc.tile_pool(name="eg", bufs=2))
            em = ctx.enter_context(tc.tile_pool(name="em", bufs=2))
            og = ctx.enter_context(tc.tile_pool(name="og", bufs=2))
            p_acc = ctx.enter_context(
                tc.tile_pool(name="p_acc", bufs=2, space="PSUM"))
            p_zt = ctx.enter_context(
                tc.tile_pool(name="p_zt", bufs=2, space="PSUM"))
            p_mm = ctx.enter_context(
                tc.tile_pool(name="p_mm", bufs=2, space="PSUM"))
            p_z2 = ctx.enter_context(
                tc.tile_pool(name="p_z2", bufs=2, space="PSUM"))

            ident = const.tile([128, 128], BF16)
            make_identity(nc, ident[:])

            _cc = [0]

            def load_const(ap, shape, dt):
                _cc[0] += 1
                t = const.tile(shape, dt, tag=f"const{_cc[0]}",
                               name=f"const{_cc[0]}")
                nc.sync.dma_start(t[:], ap)
                return t

            atom_emb_s = load_const(atom_emb[:], [128, 9, 512], BF16)
            bond_emb_s = load_const(bond_emb[:], [24, 512], BF16)
            ohb_s = load_const(ohb[:], [24, 512], BF16)
            aw1_s = load_const(aw1[:], [128, 4, 512], BF16)
            aw2_s = load_const(aw2[:], [128, 4, 512], BF16)
            bw1_s = load_const(bw1[:], [128, 4, 512], BF16)
            bw2_s = load_const(bw2[:], [128, 4, 512], BF16)
            ab1_s = load_const(ab1[:], [128, 4], F32)
            ab2_s = load_const(ab2[:], [128, 4], F32)
            bb1_s = load_const(bb1[:], [128, 4], F32)
            bb2_s = load_const(bb2[:], [128, 4], F32)
            cb1_s = [load_const(cb1[l], [128, 4], F32) for l in range(L)]
            cb2_s = [load_const(cb2[l], [128, 4], F32) for l in range(L)]
            if not ln_ident:
                cln_g_s = [load_const(cln[l, 0], [128, 512], F32)
                           for l in range(L)]
                cln_b_s = [load_const(cln[l, 1], [128, 512], F32)
                           for l in range(L)]
            idxh_s = load_const(idxh[:], [128, TS_TOT * 8], I16)
            idxe_s = load_const(idxe[:], [128, TS_TOT * 8], I16)

            hin = {}   # persistent per-seg residual tiles

            def mlp_block(rows, W, w1_s, b1_s, w2_s, b2_s, act1, evac):
                nt = W // 128
                zT = sb.tile([128, 4, W], BF16, tag=f"mT{W}", bufs=4)
                for d in range(4):
                    ztp = p_zt.tile([128, W], BF16, tag="ztp")
                    for s in range(nt):
                        nc.tensor.transpose(ztp[:, s * 128:(s + 1) * 128],
                                            rows[s][:, d * 128:(d + 1) * 128],
                                            ident[:])
                    nc.scalar.activation(zT[:, d, :], ztp[:], AF.Copy)
                a1 = sb.tile([128, 4, W], BF16, tag=f"mT{W}", bufs=4)
                for mc in range(4):
                    mm = p_mm.tile([128, W], F32, tag="mm")
                    for kc in range(4):
                        nc.tensor.matmul(mm[:],
                                         w1_s[:, kc, mc * 128:(mc + 1) * 128],
                                         zT[:, kc, :],
                                         start=(kc == 0), stop=(kc == 3))
                    nc.scalar.activation(a1[:, mc, :], mm[:], act1,
                                         bias=b1_s[:, mc:mc + 1])
                z2T = sb.tile([128, 4, W], BF16, tag=f"mT{W}", bufs=4)
                for mc in range(4):
                    mm = p_mm.tile([128, W], F32, tag="mm")
                    for kc in range(4):
                        nc.tensor.matmul(mm[:],
                                         w2_s[:, kc, mc * 128:(mc + 1) * 128],
                                         a1[:, kc, :],
                                         start=(kc == 0), stop=(kc == 3))
                    nc.scalar.activation(z2T[:, mc, :], mm[:], AF.Identity,
                                         bias=b2_s[:, mc:mc + 1])
                for s in range(nt):
                    z2p = p_z2.tile([128, 512], BF16, tag="z2p")
                    for d in range(4):
                        nc.tensor.transpose(z2p[:, d * 128:(d + 1) * 128],
                                            z2T[:, d, s * 128:(s + 1) * 128],
                                            ident[:])
                    evac(s, z2p)

            def rstd_nmrs(rsum, ssq, G):
                """LayerNorm 1/std and -mean/std via DVE-only fast rsqrt."""
                mean = sb.tile([128, 4], F32, tag="mean")
                nc.vector.tensor_scalar_mul(mean[:, :G], rsum[:, :G], 1.0 / 512)
                t1 = sb.tile([128, 4], F32, tag="t1")
                nc.vector.tensor_scalar(t1[:, :G], ssq[:, :G], 1.0 / 512,
                                        LN_EPS, op0=ALU.mult, op1=ALU.add)
                m2 = sb.tile([128, 4], F32, tag="m2")
                nc.vector.tensor_mul(m2[:, :G], mean[:, :G], mean[:, :G])
                v = sb.tile([128, 4], F32, tag="v")
                nc.vector.scalar_tensor_tensor(v[:, :G], m2[:, :G], -1.0,
                                               t1[:, :G],
                                               op0=ALU.mult, op1=ALU.add)
                vh = sb.tile([128, 4], F32, tag="vh")
                nc.vector.tensor_scalar_mul(vh[:, :G], v[:, :G], 0.5)
                y0 = sb.tile([128, 4], F32, tag="y0")
                nc.vector.tensor_scalar(
                    y0[:, :G].bitcast(I32), v[:, :G].bitcast(I32), 1, None,
                    op0=ALU.arith_shift_right)
                nc.vector.tensor_scalar(
                    y0[:, :G].bitcast(I32), y0[:, :G].bitcast(I32),
                    -1, RSQRT_MAGIC, op0=ALU.mult, op1=ALU.add)
                s2 = sb.tile([128, 4], F32, tag="s2")
                nc.vector.tensor_mul(s2[:, :G], y0[:, :G], y0[:, :G])
                u = sb.tile([128, 4], F32, tag="u")
                nc.vector.tensor_mul(u[:, :G], s2[:, :G], vh[:, :G])
                a = sb.tile([128, 4], F32, tag="a")
                nc.vector.tensor_mul(a[:, :G], y0[:, :G], u[:, :G])
                b15 = sb.tile([128, 4], F32, tag="b15")
                nc.vector.tensor_scalar_mul(b15[:, :G], y0[:, :G], 1.5)
                rstd = sb.tile([128, 4], F32, tag="rstd")
                nc.vector.scalar_tensor_tensor(rstd[:, :G], a[:, :G], -1.0,
                                               b15[:, :G],
                                               op0=ALU.mult, op1=ALU.add)
                nmrs = sb.tile([128, 4], F32, tag="nmrs")
                nc.vector.scalar_tensor_tensor(nmrs[:, :G], mean[:, :G], -1.0,
                                               rstd[:, :G],
                                               op0=ALU.mult, op1=ALU.mult)
                return rstd, nmrs

            # ================= PHASE 1: bond table (fp8) =================
            bond_rows = []
            rsum_b = sb.tile([128, 4], F32, tag="rsum")
            ssq_b = sb.tile([128, 4], F32, tag="ssq")
            for t in range(4):
                acc = p_acc.tile([128, 512], F32, tag="acc")
                nc.tensor.matmul(acc[:], ohb_s[:, t * 128:(t + 1) * 128],
                                 bond_emb_s[:], start=True, stop=True)
                rows = sb.tile([128, 512], F32, tag="rows32", bufs=6)
                nc.scalar.activation(rows[:], acc[:], AF.Identity,
                                     accum_out=rsum_b[:, t:t + 1])
                sq = sb.tile([128, 512], BF16, tag="sq")
                nc.scalar.activation(sq[:], rows[:], AF.Square,
                                     accum_out=ssq_b[:, t:t + 1])
                bond_rows.append(rows)

            rstd_b, nmrs_b = rstd_nmrs(rsum_b, ssq_b, 4)
            xhat_b = []
            for i in range(4):
                xh = sb.tile([128, 512], BF16, tag="ln16", bufs=6,
                             name=f"bxh{i}")
                nc.scalar.activation(xh[:], bond_rows[i][:], AF.Identity,
                                     scale=rstd_b[:, i:i + 1],
                                     bias=nmrs_b[:, i:i + 1])
                xhat_b.append(xh)

            def bond_evac(s, z2p):
                eout = sb.tile([128, 512], FP8, tag="eout")
                nc.scalar.activation(eout[:], z2p[:], AF.Copy)
                nc.sync.dma_start(e_table[s * 128:(s + 1) * 128, :], eout[:])

            mlp_block(xhat_b, 512, bw1_s, bb1_s, bw2_s, bb2_s, AF.Gelu,
                      bond_evac)

            # ===== materialize e_edges (fp8) during the atom encoder =====
            NCH = len(CHUNKS)
            _emat = [0]

            def ematerialize():
                c = _emat[0]
                if c >= NCH:
                    return
                _emat[0] += 1
                t0, TL, TH = ch[c]
                TC = TL + TH
                eb = em.tile([128, T2MAX, 512], FP8, tag="em")
                nc.gpsimd.dma_gather(
                    eb[:, :TC, :], e_table[:],
                    idxe_s[:, t0 * 8:(t0 + TC) * 8],
                    TC * 128, TC * 128, 512, queue_num=1 + c % 3)
                nc.sync.dma_start(e_edges[:, t0:t0 + TC, :], eb[:, :TC, :])

            # ================= PHASE 2: atom encoder =================
            for grp in GROUPS:
                ematerialize()
                ematerialize()
                W = len(grp) * 128
                G = len(grp)
                rsum = sb.tile([128, 4], F32, tag="rsum")
                ssq = sb.tile([128, 4], F32, tag="ssq")
                rows_f = []
                for i, t in enumerate(grp):
                    oh = sb.tile([128, 9, 128], BF16, tag="oha", bufs=2)
                    nc.sync.dma_start(oh[:], oha[:, t, :, :])
                    acc = p_acc.tile([128, 512], F32, tag="acc")
                    for f in range(9):
                        nc.tensor.matmul(acc[:], oh[:, f, :],
                                         atom_emb_s[:, f, :],
                                         start=(f == 0), stop=(f == 8))
                    rows = sb.tile([128, 512], F32, tag="rows32", bufs=6)
                    nc.scalar.activation(rows[:], acc[:], AF.Identity,
                                         accum_out=rsum[:, i:i + 1])
                    sq = sb.tile([128, 512], BF16, tag="sq")
                    nc.scalar.activation(sq[:], rows[:], AF.Square,
                                         accum_out=ssq[:, i:i + 1])
                    rows_f.append(rows)
                rstd, nmrs = rstd_nmrs(rsum, ssq, G)
                lnr = []
                for i in range(G):
                    xh = sb.tile([128, 512], BF16, tag="ln16", bufs=6,
                                 name=f"axh{i}")
                    nc.scalar.activation(xh[:], rows_f[i][:], AF.Identity,
                                         scale=rstd[:, i:i + 1],
                                         bias=nmrs[:, i:i + 1])
                    lnr.append(xh)

                def atom_evac(i, z2p, grp=grp):
                    t = grp[i]
                    ht = hres.tile([128, 512], BF16, tag=f"hin{t}",
                                   name=f"hin{t}")
                    hin[t] = ht
                    nc.vector.tensor_copy(ht[:], z2p[:])
                    nc.sync.dma_start(shard[0][t * 128:(t + 1) * 128, :],
                                      ht[:])

                mlp_block(lnr, W, aw1_s, ab1_s, aw2_s, ab2_s, AF.Gelu,
                          atom_evac)
            nc.gpsimd.collective_compute(
                "AllGather", ALU.bypass, replica_groups=RG,
                ins=[shard[0][:]], outs=[h_tab[0][:]])

            # ================= PHASE 3: conv layers =================
            for l in range(L):
                tab = h_tab[l % 2]
                shd = shard[(l + 1) % 2]
                w1_s = wpool.tile([128, 4, 512], BF16, tag="w1")
                nc.sync.dma_start(w1_s[:], cw1[l])
                w2_s = wpool.tile([128, 4, 512], BF16, tag="w2")
                nc.sync.dma_start(w2_s[:], cw2[l])

                pend = {}

                def issue_chunk(c, pend=pend, tab=tab):
                    t0, TL, TH = ch[c]
                    TC = TL + TH
                    hb = hg.tile([128, T2MAX, 512], BF16, tag="hb")
                    if TL:
                        nc.gpsimd.dma_gather(
                            hb[:, :TL, :], tab[:HALF, :],
                            idxh_s[:, t0 * 8:(t0 + TL) * 8],
                            TL * 128, TL * 128, 512, queue_num=1 + c % 3)
                    if TH:
                        nc.gpsimd.dma_gather(
                            hb[:, TL:TC, :], tab[HALF:, :],
                            idxh_s[:, (t0 + TL) * 8:(t0 + TC) * 8],
                            TH * 128, TH * 128, 512, queue_num=1 + c % 3)
                    eb = eg.tile([128, T2MAX, 512], BF16, tag="eb")
                    nc.gpsimd.dma_start(eb[:, :TC, :],
                                        e_edges[:, t0:t0 + TC, :])
                    oh = og.tile([128, T2MAX, 128], BF16, tag="oh")
                    nc.sync.dma_start(oh[:, :TC, :], ohe[:, t0:t0 + TC, :])
                    pend[c] = (hb, eb, oh)

                issue_chunk(0)
                zs = []
                for c in range(NCH):
                    if c + 1 < NCH:
                        issue_chunk(c + 1)
                    hb, eb, oh = pend.pop(c)
                    t0, TL, TH = ch[c]
                    TC = TL + TH
                    nc.vector.tensor_add(hb[:, :TC, :], hb[:, :TC, :],
                                         eb[:, :TC, :])
                    nc.vector.tensor_scalar_max(hb[:, :TC, :], hb[:, :TC, :],
                                                0.0)
                    for s in CHUNKS[c]:
                        tl = int(lstart[s] - t0)
                        th = int(hstart[s] - t0)
                        tls = list(range(tl, tl + int(T2[s, 0]))) + \
                            list(range(th, th + int(T2[s, 1])))
                        agg = p_acc.tile([128, 512], F32, tag="acc")
                        for i, tt in enumerate(tls):
                            nc.tensor.matmul(agg[:], oh[:, tt, :],
                                             hb[:, tt, :],
                                             start=(i == 0),
                                             stop=(i == len(tls) - 1))
                        z = sb.tile([128, 512], BF16, tag="ln16", bufs=6)
                        nc.vector.tensor_add(z[:], agg[:], hin[s][:])
                        zs.append((s, z))

                    if len(zs) >= 4 or c == NCH - 1:
                        grp = [s for s, _ in zs]
                        z_rows = [zz for _, zz in zs]
                        zs = []
                        G = len(grp)
                        W = G * 128
                        rsum = sb.tile([128, 4], F32, tag="rsum")
                        ssq = sb.tile([128, 4], F32, tag="ssq")
                        r_tiles = []

                        def conv_evac(i, z2p, grp=grp, rsum=rsum, ssq=ssq,
                                      r_tiles=r_tiles):
                            g2 = sb.tile([128, 512], F32, tag="g2", bufs=2)
                            nc.scalar.activation(g2[:], z2p[:], AF.Gelu)
                            r = sb.tile([128, 512], F32, tag="rows32", bufs=6)
                            nc.vector.scalar_tensor_tensor(
                                r[:], g2[:], 0.0, hin[grp[i]][:],
                                op0=ALU.bypass, op1=ALU.add,
                                accum_out=rsum[:, i:i + 1])
                            sq = sb.tile([128, 512], BF16, tag="sq")
                            nc.scalar.activation(sq[:], r[:], AF.Square,
                                                 accum_out=ssq[:, i:i + 1])
                            r_tiles.append(r)

                        mlp_block(z_rows, W, w1_s, cb1_s[l], w2_s,
                                  cb2_s[l], AF.Relu, conv_evac)

                        rstd, nmrs = rstd_nmrs(rsum, ssq, G)
                        for i, s in enumerate(grp):
                            rs = slice(s * 128, (s + 1) * 128)
                            if ln_ident:
                                if l == L - 1:
                                    xn = sb.tile([128, 512], F32, tag="xn")
                                    nc.scalar.activation(
                                        xn[:], r_tiles[i][:], AF.Identity,
                                        scale=rstd[:, i:i + 1],
                                        bias=nmrs[:, i:i + 1])
                                    nc.sync.dma_start(out_h[rs, :], xn[:])
                                else:
                                    nc.scalar.activation(
                                        hin[s][:], r_tiles[i][:], AF.Identity,
                                        scale=rstd[:, i:i + 1],
                                        bias=nmrs[:, i:i + 1])
                                    nc.sync.dma_start(shd[rs, :], hin[s][:])
                                continue
                            xn = sb.tile([128, 512], F32, tag="xn")
                            nc.scalar.activation(xn[:], r_tiles[i][:],
                                                 AF.Identity,
                                                 scale=rstd[:, i:i + 1],
                                                 bias=nmrs[:, i:i + 1])
                            y = sb.tile([128, 512], F32, tag="y")
                            nc.vector.tensor_mul(y[:], xn[:], cln_g_s[l][:])
                            hf_ = sb.tile([128, 512], F32, tag="hf")
                            nc.vector.tensor_add(hf_[:], y[:], cln_b_s[l][:])
                            if l == L - 1:
                                nc.sync.dma_start(out_h[rs, :], hf_[:])
                            else:
                                nc.vector.tensor_copy(hin[s][:], hf_[:])
                                nc.sync.dma_start(shd[rs, :], hin[s][:])
                if l < L - 1:
                    nc.gpsimd.collective_compute(
                        "AllGather", ALU.bypass, replica_groups=RG,
                        ins=[shd[:]], outs=[h_tab[(l + 1) % 2][:]])

    nc.compile()
    return nc


def kernel(x, edge_attr, edge_index,
           atom_emb, atom_ln_g, atom_ln_b, atom_w1, atom_b1, atom_w2, atom_b2,
           bond_emb, bond_ln_g, bond_ln_b, bond_w1, bond_b1, bond_w2, bond_b2,
           conv_w1, conv_b1, conv_w2, conv_b2, ln_g, ln_b):
    prep = _host_prep(x, edge_attr, edge_index)

    ln_ident = bool(np.all(np.asarray(ln_g) == 1.0)
                    and np.all(np.asarray(ln_b) == 0.0))
    key = (prep["TS_TOT"], tuple(prep["T2"].ravel().tolist()), ln_ident)
    if key not in _cache:
        _cache[key] = build_program(prep, ln_ident)
    nc = _cache[key]

    # fold the encoder LayerNorm affine into the first Linear
    f32 = np.float32
    aw1_abs = np.asarray(atom_ln_g, f32)[:, None] * np.asarray(atom_w1, f32)
    ab1_abs = np.asarray(atom_b1, f32) + \
        np.asarray(atom_ln_b, f32) @ np.asarray(atom_w1, f32)
    bw1_abs = np.asarray(bond_ln_g, f32)[:, None] * np.asarray(bond_w1, f32)
    bb1_abs = np.asarray(bond_b1, f32) + \
        np.asarray(bond_ln_b, f32) @ np.asarray(bond_w1, f32)

    shared = dict(
        ohb=prep["ohb"],
        atom_emb=np.ascontiguousarray(
            np.asarray(atom_emb, f32).transpose(1, 0, 2)).astype(bf),
        bond_emb=np.asarray(bond_emb, f32).reshape(24, 512).astype(bf),
        aw1=_w_sb_layout(aw1_abs), aw2=_w_sb_layout(atom_w2),
        bw1=_w_sb_layout(bw1_abs), bw2=_w_sb_layout(bond_w2),
        cw1=np.stack([_w_sb_layout(conv_w1[l]) for l in range(L)]),
        cw2=np.stack([_w_sb_layout(conv_w2[l]) for l in range(L)]),
        ab1=_b_layout(ab1_abs), ab2=_b_layout(atom_b2),
        bb1=_b_layout(bb1_abs), bb2=_b_layout(bond_b2),
        cb1=np.stack([_b_layout(conv_b1[l]) for l in range(L)]),
        cb2=np.stack([_b_layout(conv_b2[l]) for l in range(L)]),
        cln=np.stack([np.stack([_repl(ln_g[l]), _repl(ln_b[l])])
                      for l in range(L)]),
    )
    in_maps = []
    for k in range(NCORES):
        m = dict(shared)
        m["idxh"] = prep["idxh"][k]
        m["idxe"] = prep["idxe"][k]
        m["ohe"] = prep["ohe"][k]
        m["oha"] = prep["oha"][k]
        in_maps.append(m)

    res = run_bass_kernel_spmd(nc, in_maps, list(range(NCORES)))
    kernel._last_results = res
    out = np.empty((N, D), np.float32)
    for k in range(NCORES):
        out[k * NPC:(k + 1) * NPC] = np.asarray(
            res.results[k]["out_h"], np.float32)[:NPC]
    return out


# revision 24
# speedup vs baseline: 1.2058x; 1.0186x over previous
"""Trainium2 Bass kernel for a 4-layer GINE graph encoder (GNN message passing).

Strategy (8 NeuronCores, SPMD):
  - Nodes sharded: core k owns rows [k*6250, (k+1)*6250), padded to 6272 (=49*128).
  - Edges partitioned by dst owner, sorted by dst, grouped into 128-dst
    segments; segment-sum on the tensor engine with host-built one-hot
    matrices (no scatter).
  - h[src] gathered via batched dma_gather (int16 indices, low/high table
    halves as two bases) from a replicated bf16 node table in DRAM,
    rebuilt each layer with an AllGather collective.
  - Bond encoder collapsed to a 512-row fp8 table (8^3 feature combos);
    per-edge bond vectors gathered per layer with dma_gather as well.
  - Residual/h_in kept resident in SBUF across all layers.
  - Atom embedding sums via one-hot matmuls.
  - LayerNorm rstd via a DVE-only bit-trick rsqrt; encoder LN affine folded
    into the following Linear on the host.
"""
import numpy as np
import ml_dtypes
from contextlib import ExitStack

import concourse.bass as bass
import concourse.tile as tile
from concourse import bacc, mybir
from concourse.bass_utils import run_bass_kernel_spmd
from concourse.masks import make_identity

BF16 = mybir.dt.bfloat16
FP8 = mybir.dt.float8e4
F32 = mybir.dt.float32
I32 = mybir.dt.int32
I16 = mybir.dt.int16
AF = mybir.ActivationFunctionType
ALU = mybir.AluOpType
bf = ml_dtypes.bfloat16

NCORES = 8
N, E, D, L = 50000, 160000, 512, 4
NPC = N // NCORES          # 6250 real nodes per core
NT = 49                    # node tiles per core
NPAD = NT * 128            # 6272 padded nodes per core
NTOT = NCORES * NPAD       # 50176
NSEG = NT                  # 49 segments of 128 dst slots per core
PSPLIT = 25                # table region A = segs [0,25), B = segs [25,49)
ROWS_A = PSPLIT * 128      # 3200 rows/core in region A
ROWS_B = NPAD - ROWS_A     # 3072 rows/core in region B
REG_A = NCORES * ROWS_A    # 25600 table rows in region A (< int16 range)
LN_EPS = 1e-5
RSQRT_MAGIC = 0x5F3759DF

_cache = {}

# MLP groups: 13 groups of <=4 segments; gather chunks: 2 segments each
GROUPS = [list(range(g * 4, min(g * 4 + 4, NT))) for g in range(13)]
CHUNKS = [list(range(c * 2, min(c * 2 + 2, NT))) for c in range((NT + 1) // 2)]


def _host_prep(x, edge_attr, edge_index):
    """Build per-core index/one-hot arrays for the chunked gather layout."""
    x = np.asarray(x)
    ea = np.asarray(edge_attr)
    ei = np.asarray(edge_index)
    src, dst = ei[0].astype(np.int64), ei[1].astype(np.int64)
    combo_all = (ea[:, 0] * 64 + ea[:, 1] * 8 + ea[:, 2]).astype(np.int64)
    sk, sn = src // NPC, src % NPC
    hf = (sn >= ROWS_A).astype(np.int64)      # source's table region
    srcg = np.where(hf == 0, sk * ROWS_A + sn,
                    REG_A + sk * ROWS_B + (sn - ROWS_A)).astype(np.int64)

    dstc = dst // NPC
    dstl = dst % NPC
    seg = dstl // 128
    m = (dstl % 128).astype(np.int64)

    gid = (dstc * NSEG + seg) * 2 + hf
    cnt = np.bincount(gid, minlength=NCORES * NSEG * 2).reshape(
        NCORES, NSEG, 2)
    T2 = ((cnt.max(0) + 127) // 128).astype(np.int64)       # [NSEG, 2]
    empty = T2.sum(1) == 0
    T2[empty, 0] = 1

    # stream layout: per 2-seg chunk: [L(s0)][L(s1)][H(s0)][H(s1)]
    lstart = np.zeros(NSEG, np.int64)   # global tile index of seg's L tiles
    hstart = np.zeros(NSEG, np.int64)
    ch = []   # per chunk: (t0, TL, TH, [(s, l_off, h_off)])
    t = 0
    for segs in CHUNKS:
        t0 = t
        TL = int(sum(T2[s, 0] for s in segs))
        TH = int(sum(T2[s, 1] for s in segs))
        off = 0
        for s in segs:
            lstart[s] = t0 + off
            off += T2[s, 0]
        for s in segs:
            hstart[s] = t0 + off
            off += T2[s, 1]
        ch.append((t0, TL, TH))
        t += TL + TH
    TS_TOT = t
    T2MAX = max(TL + TH for _, TL, TH in ch)

    # rank edges within each (core, seg, half) group
    order = np.argsort(gid, kind="stable")
    gs = gid[order]
    starts = np.searchsorted(gs, np.arange(NCORES * NSEG * 2))
    rank = np.arange(E) - starts[gs]
    so, ho, co = seg[order], hf[order], dstc[order]
    base = np.where(ho == 0, lstart[so], hstart[so])
    gt = base + rank // 128                     # global tile
    row = rank % 128
    pos = gt * 128 + row                        # stream position

    idxh = np.zeros((NCORES, TS_TOT * 128), np.int16)
    idxe = np.zeros((NCORES, TS_TOT * 128), np.int16)
    ohe = np.zeros((NCORES, 128, TS_TOT, 128), bf)
    idxh[co, pos] = (srcg[order] - ho * REG_A).astype(np.int16)
    idxe[co, pos] = combo_all[order].astype(np.int16)
    ohe[co, row, gt, m[order]] = 1

    def sb16(a):   # [NC, TS*128] -> [NC, 128, TS*8] (16-part wrap, repl x8)
        a = a.reshape(NCORES, TS_TOT * 8, 16).transpose(0, 2, 1)
        return np.ascontiguousarray(np.tile(a, (1, 8, 1)))

    xp = np.zeros((NCORES, NPAD, 9), np.int64)
    xp[:, :NPC] = x.reshape(NCORES, NPC, 9)
    oha = np.zeros((NCORES, 128, NT, 9, 128), bf)
    kk, nn, ff = np.meshgrid(np.arange(NCORES), np.arange(NPAD), np.arange(9),
                             indexing="ij")
    oha[kk.ravel(), xp.ravel(), (nn // 128).ravel(), ff.ravel(),
        (nn % 128).ravel()] = 1

    ohb = np.zeros((24, 512), bf)
    c = np.arange(512)
    ohb[(c // 64), c] = 1
    ohb[8 + (c // 8) % 8, c] = 1
    ohb[16 + c % 8, c] = 1

    return dict(T2=T2, ch=ch, lstart=lstart, hstart=hstart, TS_TOT=TS_TOT,
                T2MAX=T2MAX, idxh=sb16(idxh), idxe=sb16(idxe), ohe=ohe,
                oha=oha, ohb=ohb)


def _w_sb_layout(w):
    return np.ascontiguousarray(
        np.asarray(w, np.float32).reshape(4, 128, 512).transpose(1, 0, 2)
    ).astype(bf)


def _b_layout(b):
    return np.ascontiguousarray(
        np.asarray(b, np.float32).reshape(4, 128).T).astype(np.float32)


def _repl(v):
    return np.ascontiguousarray(
        np.broadcast_to(np.asarray(v, np.float32), (128, 512)))


def build_program(P, ln_ident):
    T2, ch, lstart, hstart = P["T2"], P["ch"], P["lstart"], P["hstart"]
    TS_TOT, T2MAX = P["TS_TOT"], P["T2MAX"]

    nc = bacc.Bacc("TRN2", target_bir_lowering=False, debug=False,
                   num_devices=NCORES, num_swdge_queues=4)

    def din(name, shape, dt):
        return nc.dram_tensor(name, shape, dt, kind="ExternalInput")

    idxh = din("idxh", [128, TS_TOT * 8], I16)
    idxe = din("idxe", [128, TS_TOT * 8], I16)
    ohe = din("ohe", [128, TS_TOT, 128], BF16)
    oha = din("oha", [128, NT, 9, 128], BF16)
    ohb = din("ohb", [24, 512], BF16)
    atom_emb = din("atom_emb", [128, 9, 512], BF16)
    bond_emb = din("bond_emb", [24, 512], BF16)
    aw1 = din("aw1", [128, 4, 512], BF16)
    aw2 = din("aw2", [128, 4, 512], BF16)
    bw1 = din("bw1", [128, 4, 512], BF16)
    bw2 = din("bw2", [128, 4, 512], BF16)
    cw1 = din("cw1", [L, 128, 4, 512], BF16)
    cw2 = din("cw2", [L, 128, 4, 512], BF16)
    ab1 = din("ab1", [128, 4], F32)
    ab2 = din("ab2", [128, 4], F32)
    bb1 = din("bb1", [128, 4], F32)
    bb2 = din("bb2", [128, 4], F32)
    cb1 = din("cb1", [L, 128, 4], F32)
    cb2 = din("cb2", [L, 128, 4], F32)
    cln = din("cln", [L, 2, 128, 512], F32)

    out_h = nc.dram_tensor("out_h", [NPAD, 512], F32, kind="ExternalOutput")

    shard = [nc.dram_tensor(f"shard{i}", [NPAD, 512], BF16) for i in range(2)]
    h_tab = [nc.dram_tensor(f"h_tab{i}", [NTOT, 512], BF16,
                            addr_space="Shared") for i in range(2)]
    e_table = nc.dram_tensor("e_table", [512, 512], FP8)
    e_edges = nc.dram_tensor("e_edges", [128, TS_TOT, 512], FP8)

    RG = [list(range(NCORES))]

    with tile.TileContext(nc) as tc:
        with ExitStack() as ctx:
            const = ctx.enter_context(tc.tile_pool(name="const", bufs=1))
            sb = ctx.enter_context(tc.tile_pool(name="sb", bufs=2))
            wpool = ctx.enter_context(tc.tile_pool(name="wpool", bufs=2))
            hres = ctx.enter_context(tc.tile_pool(name="hres", bufs=1))
            hg = ctx.enter_context(tc.tile_pool(name="hg", bufs=3))
            em = ctx.enter_context(tc.tile_pool(name="em", bufs=2))
            og = ctx.enter_context(tc.tile_pool(name="og", bufs=2))
            p_acc = ctx.enter_context(
                tc.tile_pool(name="p_acc", bufs=2, space="PSUM"))
            p_zt = ctx.enter_context(
                tc.tile_pool(name="p_zt", bufs=2, space="PSUM"))
            p_mm = ctx.enter_context(
                tc.tile_pool(name="p_mm", bufs=2, space="PSUM"))
            p_z2 = ctx.enter_context(
                tc.tile_pool(name="p_z2", bufs=2, space="PSUM"))

            ident = const.tile([128, 128], BF16)
            make_identity(nc, ident[:])

            _cc = [0]

            def load_const(ap, shape, dt):
                _cc[0] += 1
                t = const.tile(shape, dt, tag=f"const{_cc[0]}",
                               name=f"const{_cc[0]}")
                nc.sync.dma_start(t[:], ap)
                return t

            atom_emb_s = load_const(atom_emb[:], [128, 9, 512], BF16)
            bond_emb_s = load_const(bond_emb[:], [24, 512], BF16)
            ohb_s = load_const(ohb[:], [24, 512], BF16)
            aw1_s = load_const(aw1[:], [128, 4, 512], BF16)
            aw2_s = load_const(aw2[:], [128, 4, 512], BF16)
            bw1_s = load_const(bw1[:], [128, 4, 512], BF16)
            bw2_s = load_const(bw2[:], [128, 4, 512], BF16)
            ab1_s = load_const(ab1[:], [128, 4], F32)
            ab2_s = load_const(ab2[:], [128, 4], F32)
            bb1_s = load_const(bb1[:], [128, 4], F32)
            bb2_s = load_const(bb2[:], [128, 4], F32)
            cb1_s = [load_const(cb1[l], [128, 4], F32) for l in range(L)]
            cb2_s = [load_const(cb2[l], [128, 4], F32) for l in range(L)]
            if not ln_ident:
                cln_g_s = [load_const(cln[l, 0], [128, 512], F32)
                           for l in range(L)]
                cln_b_s = [load_const(cln[l, 1], [128, 512], F32)
                           for l in range(L)]
            idxh_s = load_const(idxh[:], [128, TS_TOT * 8], I16)
            idxe_s = load_const(idxe[:], [128, TS_TOT * 8], I16)

            hin = {}   # persistent per-seg residual tiles

            def mlp_block(rows, W, w1_s, b1_s, w2_s, b2_s, act1, evac):
                nt = W // 128
                zT = sb.tile([128, 4, W], BF16, tag=f"mT{W}", bufs=3)
                for d in range(4):
                    ztp = p_zt.tile([128, W], BF16, tag="ztp")
                    for s in range(nt):
                        nc.tensor.transpose(ztp[:, s * 128:(s + 1) * 128],
                                            rows[s][:, d * 128:(d + 1) * 128],
                                            ident[:])
                    nc.scalar.activation(zT[:, d, :], ztp[:], AF.Copy)
                a1 = sb.tile([128, 4, W], BF16, tag=f"mT{W}", bufs=3)
                for mc in range(4):
                    mm = p_mm.tile([128, W], F32, tag="mm")
                    for kc in range(4):
                        nc.tensor.matmul(mm[:],
                                         w1_s[:, kc, mc * 128:(mc + 1) * 128],
                                         zT[:, kc, :],
                                         start=(kc == 0), stop=(kc == 3))
                    nc.scalar.activation(a1[:, mc, :], mm[:], act1,
                                         bias=b1_s[:, mc:mc + 1])
                z2T = sb.tile([128, 4, W], BF16, tag=f"mT{W}", bufs=3)
                for mc in range(4):
                    mm = p_mm.tile([128, W], F32, tag="mm")
                    for kc in range(4):
                        nc.tensor.matmul(mm[:],
                                         w2_s[:, kc, mc * 128:(mc + 1) * 128],
                                         a1[:, kc, :],
                                         start=(kc == 0), stop=(kc == 3))
                    nc.scalar.activation(z2T[:, mc, :], mm[:], AF.Identity,
                                         bias=b2_s[:, mc:mc + 1])
                for s in range(nt):
                    z2p = p_z2.tile([128, 512], BF16, tag="z2p")
                    for d in range(4):
                        nc.tensor.transpose(z2p[:, d * 128:(d + 1) * 128],
                                            z2T[:, d, s * 128:(s + 1) * 128],
                                            ident[:])
                    evac(s, z2p)

            def rstd_nmrs(rsum, ssq, G):
                """LayerNorm 1/std and -mean/std via DVE-only fast rsqrt."""
                mean = sb.tile([128, 4], F32, tag="mean")
                nc.vector.tensor_scalar_mul(mean[:, :G], rsum[:, :G], 1.0 / 512)
                t1 = sb.tile([128, 4], F32, tag="t1")
                nc.vector.tensor_scalar(t1[:, :G], ssq[:, :G], 1.0 / 512,
                                        LN_EPS, op0=ALU.mult, op1=ALU.add)
                m2 = sb.tile([128, 4], F32, tag="m2")
                nc.vector.tensor_mul(m2[:, :G], mean[:, :G], mean[:, :G])
                v = sb.tile([128, 4], F32, tag="v")
                nc.vector.scalar_tensor_tensor(v[:, :G], m2[:, :G], -1.0,
                                               t1[:, :G],
                                               op0=ALU.mult, op1=ALU.add)
                vh = sb.tile([128, 4], F32, tag="vh")
                nc.vector.tensor_scalar_mul(vh[:, :G], v[:, :G], 0.5)
                y0 = sb.tile([128, 4], F32, tag="y0")
                nc.vector.tensor_scalar(
                    y0[:, :G].bitcast(I32), v[:, :G].bitcast(I32), 1, None,
                    op0=ALU.arith_shift_right)
                nc.vector.tensor_scalar(
                    y0[:, :G].bitcast(I32), y0[:, :G].bitcast(I32),
                    -1, RSQRT_MAGIC, op0=ALU.mult, op1=ALU.add)
                s2 = sb.tile([128, 4], F32, tag="s2")
                nc.vector.tensor_mul(s2[:, :G], y0[:, :G], y0[:, :G])
                u = sb.tile([128, 4], F32, tag="u")
                nc.vector.tensor_mul(u[:, :G], s2[:, :G], vh[:, :G])
                a = sb.tile([128, 4], F32, tag="a")
                nc.vector.tensor_mul(a[:, :G], y0[:, :G], u[:, :G])
                b15 = sb.tile([128, 4], F32, tag="b15")
                nc.vector.tensor_scalar_mul(b15[:, :G], y0[:, :G], 1.5)
                rstd = sb.tile([128, 4], F32, tag="rstd")
                nc.vector.scalar_tensor_tensor(rstd[:, :G], a[:, :G], -1.0,
                                               b15[:, :G],
                                               op0=ALU.mult, op1=ALU.add)
                nmrs = sb.tile([128, 4], F32, tag="nmrs")
                nc.vector.scalar_tensor_tensor(nmrs[:, :G], mean[:, :G], -1.0,
                                               rstd[:, :G],
                                               op0=ALU.mult, op1=ALU.mult)
                return rstd, nmrs

            # ================= PHASE 1: bond table (fp8) =================
            bond_rows = []
            rsum_b = sb.tile([128, 4], F32, tag="rsum")
            ssq_b = sb.tile([128, 4], F32, tag="ssq")
            for t in range(4):
                acc = p_acc.tile([128, 512], F32, tag="acc")
                nc.tensor.matmul(acc[:], ohb_s[:, t * 128:(t + 1) * 128],
                                 bond_emb_s[:], start=True, stop=True)
                rows = sb.tile([128, 512], F32, tag="rows32", bufs=6)
                nc.scalar.activation(rows[:], acc[:], AF.Identity,
                                     accum_out=rsum_b[:, t:t + 1])
                sq = sb.tile([128, 512], BF16, tag="sq")
                nc.scalar.activation(sq[:], rows[:], AF.Square,
                                     accum_out=ssq_b[:, t:t + 1])
                bond_rows.append(rows)

            rstd_b, nmrs_b = rstd_nmrs(rsum_b, ssq_b, 4)
            xhat_b = []
            for i in range(4):
                xh = sb.tile([128, 512], BF16, tag="ln16", bufs=6,
                             name=f"bxh{i}")
                nc.scalar.activation(xh[:], bond_rows[i][:], AF.Identity,
                                     scale=rstd_b[:, i:i + 1],
                                     bias=nmrs_b[:, i:i + 1])
                xhat_b.append(xh)

            def bond_evac(s, z2p):
                eout = sb.tile([128, 512], FP8, tag="eout")
                nc.scalar.activation(eout[:], z2p[:], AF.Copy)
                nc.sync.dma_start(e_table[s * 128:(s + 1) * 128, :], eout[:])

            mlp_block(xhat_b, 512, bw1_s, bb1_s, bw2_s, bb2_s, AF.Gelu,
                      bond_evac)

            # ===== materialize e_edges (fp8) during the atom encoder =====
            NCH = len(CHUNKS)
            _emat = [0]

            def ematerialize():
                c = _emat[0]
                if c >= NCH:
                    return
                _emat[0] += 1
                t0, TL, TH = ch[c]
                TC = TL + TH
                eb = em.tile([128, T2MAX, 512], FP8, tag="em")
                for a in range(0, TC, 8):
                    b = min(a + 8, TC)
                    nc.gpsimd.dma_gather(
                        eb[:, a:b, :], e_table[:],
                        idxe_s[:, (t0 + a) * 8:(t0 + b) * 8],
                        (b - a) * 128, (b - a) * 128, 512,
                        queue_num=1 + c % 3)
                nc.sync.dma_start(e_edges[:, t0:t0 + TC, :], eb[:, :TC, :])

            # ================= PHASE 2: atom encoder =================
            for grp in GROUPS:
                ematerialize()
                ematerialize()
                W = len(grp) * 128
                G = len(grp)
                rsum = sb.tile([128, 4], F32, tag="rsum")
                ssq = sb.tile([128, 4], F32, tag="ssq")
                rows_f = []
                for i, t in enumerate(grp):
                    oh = sb.tile([128, 9, 128], BF16, tag="oha", bufs=2)
                    nc.sync.dma_start(oh[:], oha[:, t, :, :])
                    acc = p_acc.tile([128, 512], F32, tag="acc")
                    for f in range(9):
                        nc.tensor.matmul(acc[:], oh[:, f, :],
                                         atom_emb_s[:, f, :],
                                         start=(f == 0), stop=(f == 8))
                    rows = sb.tile([128, 512], F32, tag="rows32", bufs=6)
                    nc.scalar.activation(rows[:], acc[:], AF.Identity,
                                         accum_out=rsum[:, i:i + 1])
                    sq = sb.tile([128, 512], BF16, tag="sq")
                    nc.scalar.activation(sq[:], rows[:], AF.Square,
                                         accum_out=ssq[:, i:i + 1])
                    rows_f.append(rows)
                rstd, nmrs = rstd_nmrs(rsum, ssq, G)
                lnr = []
                for i in range(G):
                    xh = sb.tile([128, 512], BF16, tag="ln16", bufs=6,
                                 name=f"axh{i}")
                    nc.scalar.activation(xh[:], rows_f[i][:], AF.Identity,
                                         scale=rstd[:, i:i + 1],
                                         bias=nmrs[:, i:i + 1])
                    lnr.append(xh)

                def atom_evac(i, z2p, grp=grp):
                    t = grp[i]
                    ht = hres.tile([128, 512], BF16, tag=f"hin{t}",
                                   name=f"hin{t}")
                    hin[t] = ht
                    nc.vector.tensor_copy(ht[:], z2p[:])
                    nc.sync.dma_start(shard[0][t * 128:(t + 1) * 128, :],
                                      ht[:])

                mlp_block(lnr, W, aw1_s, ab1_s, aw2_s, ab2_s, AF.Gelu,
                          atom_evac)
            nc.gpsimd.collective_compute(
                "AllGather", ALU.bypass, replica_groups=RG,
                ins=[shard[0][:ROWS_A, :]], outs=[h_tab[0][:REG_A, :]])
            nc.gpsimd.collective_compute(
                "AllGather", ALU.bypass, replica_groups=RG,
                ins=[shard[0][ROWS_A:, :]], outs=[h_tab[0][REG_A:, :]])

            # ================= PHASE 3: conv layers =================
            for l in range(L):
                tab = h_tab[l % 2]
                shd = shard[(l + 1) % 2]
                w1_s = wpool.tile([128, 4, 512], BF16, tag="w1")
                nc.sync.dma_start(w1_s[:], cw1[l])
                w2_s = wpool.tile([128, 4, 512], BF16, tag="w2")
                nc.sync.dma_start(w2_s[:], cw2[l])

                pend = {}

                def issue_chunk(c, pend=pend, tab=tab):
                    t0, TL, TH = ch[c]
                    TC = TL + TH
                    hb = hg.tile([128, T2MAX, 512], BF16, tag="hb")
                    if TL:
                        nc.gpsimd.dma_gather(
                            hb[:, :TL, :], tab[:REG_A, :],
                            idxh_s[:, t0 * 8:(t0 + TL) * 8],
                            TL * 128, TL * 128, 512, queue_num=1 + c % 3)
                    if TH:
                        nc.gpsimd.dma_gather(
                            hb[:, TL:TC, :], tab[REG_A:, :],
                            idxh_s[:, (t0 + TL) * 8:(t0 + TC) * 8],
                            TH * 128, TH * 128, 512, queue_num=1 + c % 3)
                    oh = og.tile([128, T2MAX, 128], BF16, tag="oh")
                    nc.sync.dma_start(oh[:, :TC, :], ohe[:, t0:t0 + TC, :])
                    pend[c] = (hb, oh)

                issue_chunk(0)
                zs = []
                for c in range(NCH):
                    if c + 1 < NCH:
                        issue_chunk(c + 1)
                    hb, oh = pend.pop(c)
                    t0, TL, TH = ch[c]
                    TC = TL + TH
                    eb = em.tile([128, T2MAX, 512], BF16, tag="ebl")
                    nc.gpsimd.dma_start(eb[:, :TC, :],
                                        e_edges[:, t0:t0 + TC, :])
                    nc.vector.tensor_add(hb[:, :TC, :], hb[:, :TC, :],
                                         eb[:, :TC, :])
                    nc.vector.tensor_scalar_max(hb[:, :TC, :], hb[:, :TC, :],
                                                0.0)
                    for s in CHUNKS[c]:
                        tl = int(lstart[s] - t0)
                        th = int(hstart[s] - t0)
                        tls = list(range(tl, tl + int(T2[s, 0]))) + \
                            list(range(th, th + int(T2[s, 1])))
                        agg = p_acc.tile([128, 512], F32, tag="acc")
                        for i, tt in enumerate(tls):
                            nc.tensor.matmul(agg[:], oh[:, tt, :],
                                             hb[:, tt, :],
                                             start=(i == 0),
                                             stop=(i == len(tls) - 1))
                        z = sb.tile([128, 512], BF16, tag="ln16", bufs=6)
                        nc.vector.tensor_add(z[:], agg[:], hin[s][:])
                        zs.append((s, z))

                    if len(zs) >= 4 or c == NCH - 1:
                        grp = [s for s, _ in zs]
                        z_rows = [zz for _, zz in zs]
                        zs = []
                        G = len(grp)
                        W = G * 128
                        rsum = sb.tile([128, 4], F32, tag="rsum")
                        ssq = sb.tile([128, 4], F32, tag="ssq")
                        r_tiles = []

                        def conv_evac(i, z2p, grp=grp, rsum=rsum, ssq=ssq,
                                      r_tiles=r_tiles):
                            g2 = sb.tile([128, 512], F32, tag="g2", bufs=2)
                            nc.scalar.activation(g2[:], z2p[:], AF.Gelu)
                            r = sb.tile([128, 512], F32, tag="rows32", bufs=6)
                            nc.vector.scalar_tensor_tensor(
                                r[:], g2[:], 0.0, hin[grp[i]][:],
                                op0=ALU.bypass, op1=ALU.add,
                                accum_out=rsum[:, i:i + 1])
                            sq = sb.tile([128, 512], BF16, tag="sq")
                            nc.scalar.activation(sq[:], r[:], AF.Square,
                                                 accum_out=ssq[:, i:i + 1])
                            r_tiles.append(r)

                        mlp_block(z_rows, W, w1_s, cb1_s[l], w2_s,
                                  cb2_s[l], AF.Relu, conv_evac)

                        rstd, nmrs = rstd_nmrs(rsum, ssq, G)
                        for i, s in enumerate(grp):
                            rs = slice(s * 128, (s + 1) * 128)
                            if ln_ident:
                                if l == L - 1:
                                    xn = sb.tile([128, 512], F32, tag="xn")
                                    nc.scalar.activation(
                                        xn[:], r_tiles[i][:], AF.Identity,
                                        scale=rstd[:, i:i + 1],
                                        bias=nmrs[:, i:i + 1])
                                    nc.sync.dma_start(out_h[rs, :], xn[:])
                                else:
                                    nc.scalar.activation(
                                        hin[s][:], r_tiles[i][:], AF.Identity,
                                        scale=rstd[:, i:i + 1],
                                        bias=nmrs[:, i:i + 1])
                                    nc.sync.dma_start(shd[rs, :], hin[s][:])
                                continue
                            xn = sb.tile([128, 512], F32, tag="xn")
                            nc.scalar.activation(xn[:], r_tiles[i][:],
                                                 AF.Identity,
                                                 scale=rstd[:, i:i + 1],
                                                 bias=nmrs[:, i:i + 1])
                            y = sb.tile([128, 512], F32, tag="y")
                            nc.vector.tensor_mul(y[:], xn[:], cln_g_s[l][:])
                            hf_ = sb.tile([128, 512], F32, tag="hf")
                            nc.vector.tensor_add(hf_[:], y[:], cln_b_s[l][:])
                            if l == L - 1:
                                nc.sync.dma_start(out_h[rs, :], hf_[:])
                            else:
                                nc.vector.tensor_copy(hin[s][:], hf_[:])
                                nc.sync.dma_start(shd[rs, :], hin[s][:])
                if l < L - 1:
                    nc.gpsimd.collective_compute(
                        "AllGather", ALU.bypass, replica_groups=RG,
                        ins=[shd[:ROWS_A, :]],
                        outs=[h_tab[(l + 1) % 2][:REG_A, :]])
                    nc.gpsimd.collective_compute(
                        "AllGather", ALU.bypass, replica_groups=RG,
                        ins=[shd[ROWS_A:, :]],
                        outs=[h_tab[(l + 1) % 2][REG_A:, :]])

    nc.compile()
    return nc


def kernel(x, edge_attr, edge_index,
           atom_emb, atom_ln_g, atom_ln_b, atom_w1, atom_b1, atom_w2, atom_b2,
           bond_emb, bond_ln_g, bond_ln_b, bond_w1, bond_b1, bond_w2, bond_b2,
           conv_w1, conv_b1, conv_w2, conv_b2, ln_g, ln_b):
    prep = _host_prep(x, edge_attr, edge_index)

    ln_ident = bool(np.all(np.asarray(ln_g) == 1.0)
                    and np.all(np.asarray(ln_b) == 0.0))
    key = (prep["TS_TOT"], tuple(prep["T2"].ravel().tolist()), ln_ident)
    if key not in _cache:
        _cache[key] = build_program(prep, ln_ident)
    nc = _cache[key]

    # fold the encoder LayerNorm affine into the first Linear
    f32 = np.float32
    aw1_abs = np.asarray(atom_ln_g, f32)[:, None] * np.asarray(atom_w1, f32)
    ab1_abs = np.asarray(atom_b1, f32) + \
        np.asarray(atom_ln_b, f32) @ np.asarray(atom_w1, f32)
    bw1_abs = np.asarray(bond_ln_g, f32)[:, None] * np.asarray(bond_w1, f32)
    bb1_abs = np.asarray(bond_b1, f32) + \
        np.asarray(bond_ln_b, f32) @ np.asarray(bond_w1, f32)

    shared = dict(
        ohb=prep["ohb"],
        atom_emb=np.ascontiguousarray(
            np.asarray(atom_emb, f32).transpose(1, 0, 2)).astype(bf),
        bond_emb=np.asarray(bond_emb, f32).reshape(24, 512).astype(bf),
        aw1=_w_sb_layout(aw1_abs), aw2=_w_sb_layout(atom_w2),
        bw1=_w_sb_layout(bw1_abs), bw2=_w_sb_layout(bond_w2),
        cw1=np.stack([_w_sb_layout(conv_w1[l]) for l in range(L)]),
        cw2=np.stack([_w_sb_layout(conv_w2[l]) for l in range(L)]),
        ab1=_b_layout(ab1_abs), ab2=_b_layout(atom_b2),
        bb1=_b_layout(bb1_abs), bb2=_b_layout(bond_b2),
        cb1=np.stack([_b_layout(conv_b1[l]) for l in range(L)]),
        cb2=np.stack([_b_layout(conv_b2[l]) for l in range(L)]),
        cln=np.stack([np.stack([_repl(ln_g[l]), _repl(ln_b[l])])
                      for l in range(L)]),
    )
    in_maps = []
    for k in range(NCORES):
        m = dict(shared)
        m["idxh"] = prep["idxh"][k]
        m["idxe"] = prep["idxe"][k]
        m["ohe"] = prep["ohe"][k]
        m["oha"] = prep["oha"][k]
        in_maps.append(m)

    res = run_bass_kernel_spmd(nc, in_maps, list(range(NCORES)))
    kernel._last_results = res
    out = np.empty((N, D), np.float32)
    for k in range(NCORES):
        out[k * NPC:(k + 1) * NPC] = np.asarray(
            res.results[k]["out_h"], np.float32)[:NPC]
    return out
